# revision 1
# baseline (speedup 1.0000x reference)
"""ExpertLinear (dense MoE blend) Trainium2 kernel.

y[b,o] = sum_k ew[b,k] * (x[b,:] @ W[k,o,:]) + sum_k ew[b,k] * bias[k,o]

Data-parallel over B across 8 cores; each core streams the whole blended
weight tensor. Layout/precision choices:
  - Host pre-transposes W -> wT[k, i, o] (contraction dim i on partitions,
    fully contiguous per-partition DMA rows) and casts it to bf16, with 32
    zero columns appended per row block. bf16 halves the dominant HBM
    stream (32MB -> ~17MB per core) and - critically - lets all 16 weight
    tiles stay live in SBUF at once: no buffer reuse means no DMA needs
    both a WAW and WAR wait, which matters because this walrus build
    accepts at most ONE sync wait per instruction.
  - All small operands (xT i-tiles, ew columns replicated across
    partitions, ewT, bias) are packed host-side into one fp32 tensor `xe`
    and arrive via a single DMA (single semaphore lane).
  - VectorE pre-scales xs_k[i,b] = x[b,i] * ew[b,k] into bf16; the PE then
    accumulates the fp32 bias matmul (ewT.T @ bias, K=8) plus all 128
    bf16 W matmuls into 2 PSUM banks, evicted once at the end.
  - Per W tile, a zero-result matmul (wt-slice.T @ zero-column) absorbs
    the tile's DMA-lane wait on its own PE instruction, so the real
    matmuls carry at most their single DVE-tick wait.
Per-core HBM traffic ~= 18.5 MB; measured numerics ~2e-3 relative L2.
"""

import numpy as np

B, E, IN, OUT = 512, 8, 1024, 1024
NCORES = 8
BL = B // NCORES  # 64 rows per core
P = 128
NIT = IN // P  # 8 i-tiles
W_DMA_ITILES = 16  # i-tiles per W DMA
OUTP = OUT + 32  # zero-padded row length in the bf16 W stream
NTILES = (E * NIT) // W_DMA_ITILES  # 16 weight tiles, all live in SBUF

# xe column layout (float32, 128 partitions)
XT_C = 0                       # 8 i-tiles of xT: [128, 8*64]
EWB_C = XT_C + NIT * BL        # ew columns replicated: [128, 8*64]
EWT_C = EWB_C + E * BL         # ewT on partitions 0..7: [8, 64]
BIAS_C = EWT_C + BL            # bias on partitions 0..7: [8, 1024]
XE_COLS = BIAS_C + OUT

_compiled = None


def _patch_drain_split():
    """The walrus build in this container rejects any instruction carrying
    more than one sync wait, including the kernel-tail Drain that
    TileContext emits with one wait per active semaphore. Split it into a
    sequence of single-wait drains (sequencer-FIFO keeps them ordered;
    the set of waits is identical)."""
    import concourse.tile as tile_mod

    if getattr(tile_mod.TileContext, "_drain_split_patched", False):
        return
    from concourse.tile_sem_assignment import N_PROCS
    from concourse.vector_clock import ScopedClock, VectorClock

    def _drain_and_barrier(self, tick_clock, wait_clock):
        gc = tick_clock.global_clock
        for p in range(N_PROCS):
            t = gc[p]
            if t <= 0:
                continue
            ticks = [0] * N_PROCS
            ticks[p] = t
            di = self.nc.sync.drain()
            wait_clock.add_sem_waits(
                di.ins, ScopedClock({None: VectorClock(ticks)})
            )
        self.nc.all_engine_barrier()
        assert self.sems is not None
        popped = self.nc._tile_sem_poison_stack.pop()
        assert popped is self._sem_poison
        self.nc.clear_and_free_semaphores(list(self.sems.allocated().values()))
        self.nc.all_engine_barrier()

    tile_mod.TileContext._drain_and_barrier = _drain_and_barrier
    tile_mod.TileContext._drain_split_patched = True


def _build():
    import concourse.bass as bass
    import concourse.mybir as mybir
    import concourse.tile as tile

    _patch_drain_split()

    f32 = mybir.dt.float32
    bf16 = mybir.dt.bfloat16

    nc = bass.Bass()
    xe_d = nc.dram_tensor("xe", [P, XE_COLS], f32, kind="ExternalInput")
    wT_d = nc.dram_tensor("wT", [E, IN, OUTP], bf16, kind="ExternalInput")
    y_d = nc.dram_tensor("y", [BL, OUT], f32, kind="ExternalOutput")

    with tile.TileContext(nc) as tc:
        with (
            tc.tile_pool(name="const", bufs=1) as const,
            tc.tile_pool(name="wpool", bufs=1) as wpool,
            tc.tile_pool(name="psum", bufs=2, space="PSUM") as psum,
        ):
            xe = const.tile([P, XE_COLS], f32)
            xs = const.tile([P, E * NIT * BL], bf16)
            y_sb = const.tile([BL, OUT], f32)
            wts = [
                wpool.tile([P, W_DMA_ITILES * OUTP], bf16,
                           name=f"wt{t}", tag=f"wt{t}")
                for t in range(NTILES)
            ]

            nc.sync.dma_start(xe[:], xe_d[:])

            # xs_k[i, b] = xT[i, b] * ew[b, k], downcast to bf16
            for k in range(E):
                for ib in range(NIT):
                    nc.vector.tensor_tensor(
                        xs[:, (k * NIT + ib) * BL:(k * NIT + ib + 1) * BL],
                        xe[:, XT_C + ib * BL:XT_C + (ib + 1) * BL],
                        xe[:, EWB_C + k * BL:EWB_C + (k + 1) * BL],
                        mybir.AluOpType.mult,
                    )

            ps0 = psum.tile([BL, 512], f32)
            ps1 = psum.tile([BL, 512], f32)
            ewt_ap = xe[0:E, EWT_C:EWT_C + BL]
            # bias term: y += ewT.T @ bias (K=8, fp32 matmul - only 2 of them)
            nc.tensor.matmul(
                ps0[:], ewt_ap, xe[0:E, BIAS_C:BIAS_C + 512],
                start=True, stop=False,
            )
            nc.tensor.matmul(
                ps1[:], ewt_ap, xe[0:E, BIAS_C + 512:BIAS_C + 1024],
                start=True, stop=False,
            )

            # wT viewed as a flat stream of E*NIT [128, OUTP] i-blocks,
            # grouped W_DMA_ITILES per DMA/tile.
            wT_flat = wT_d[:].rearrange("k (n p) o -> (k n) p o", p=P)
            for t in range(NTILES):
                wt = wts[t]
                src = wT_flat[t * W_DMA_ITILES:(t + 1) * W_DMA_ITILES].rearrange(
                    "n p o -> p n o"
                )
                dst = wt[:].rearrange("p (n o) -> p n o", n=W_DMA_ITILES)
                nc.sync.dma_start(dst, src)
                # zero matmul: wt-slice.T @ zero-column adds 0 to ps0 but
                # absorbs this tile's DMA-lane wait on its own PE
                # instruction (one-sync-wait walrus limit); its ready-set
                # is a subset of the real matmuls' and its priority is
                # earlier, so it schedules first.
                nc.tensor.matmul(
                    ps0[:, 0:1],
                    wt[:, 0:BL],
                    wt[:, OUT:OUT + 1],
                    start=False, stop=False,
                )
                for j in range(W_DMA_ITILES):
                    blk = t * W_DMA_ITILES + j   # global i-block = k*NIT+ib
                    lhsT = xs[:, blk * BL:(blk + 1) * BL]
                    last = blk == E * NIT - 1
                    nc.tensor.matmul(
                        ps0[:], lhsT,
                        wt[:, j * OUTP:j * OUTP + 512],
                        start=False, stop=last,
                    )
                    nc.tensor.matmul(
                        ps1[:], lhsT,
                        wt[:, j * OUTP + 512:j * OUTP + 1024],
                        start=False, stop=last,
                    )

            nc.vector.tensor_copy(y_sb[:, 0:512], ps0[:])
            nc.vector.tensor_copy(y_sb[:, 512:1024], ps1[:])
            nc.sync.dma_start(y_d[:], y_sb[:])

    return nc


def _get_compiled():
    global _compiled
    if _compiled is None:
        _compiled = _build()
    return _compiled


_wT_cache = None


def _make_in_maps(x, expert_weights, weight, bias):
    global _wT_cache
    import ml_dtypes

    if _wT_cache is None or _wT_cache[0] is not weight:
        wT = np.zeros((E, IN, OUTP), dtype=ml_dtypes.bfloat16)
        wT[:, :, :OUT] = (
            np.asarray(weight, dtype=np.float32)
            .transpose(0, 2, 1)
            .astype(ml_dtypes.bfloat16)
        )
        _wT_cache = (weight, wT)
    wT = _wT_cache[1]
    bias = np.ascontiguousarray(np.asarray(bias, dtype=np.float32))
    x = np.asarray(x, dtype=np.float32)
    ew = np.asarray(expert_weights, dtype=np.float32)
    in_maps = []
    for c in range(NCORES):
        xl = x[c * BL:(c + 1) * BL]          # [64, IN]
        ewl = ew[c * BL:(c + 1) * BL]        # [64, E]
        xe = np.zeros((P, XE_COLS), dtype=np.float32)
        xT = xl.T.reshape(NIT, P, BL)        # [8, 128, 64]
        xe[:, XT_C:XT_C + NIT * BL] = xT.transpose(1, 0, 2).reshape(P, NIT * BL)
        ewb = np.broadcast_to(ewl.T[:, None, :], (E, P, BL))  # [8, 128, 64]
        xe[:, EWB_C:EWB_C + E * BL] = ewb.transpose(1, 0, 2).reshape(P, E * BL)
        xe[0:E, EWT_C:EWT_C + BL] = ewl.T
        xe[0:E, BIAS_C:BIAS_C + OUT] = bias
        in_maps.append({"xe": xe, "wT": wT})
    return in_maps


def kernel(x, expert_weights, weight, bias, _trace=False):
    from concourse.bass_utils import run_bass_kernel_spmd

    nc = _get_compiled()
    in_maps = _make_in_maps(x, expert_weights, weight, bias)
    res = run_bass_kernel_spmd(
        nc, in_maps, core_ids=list(range(NCORES)), trace=_trace
    )
    y = np.concatenate([r["y"] for r in res.results], axis=0).astype(np.float32)
    if _trace:
        return y, res
    return y



# revision 3
# speedup vs baseline: 2.1601x; 2.1601x over previous
"""ExpertLinear (dense MoE blend) Trainium2 kernel — expert-parallel.

y[b,o] = sum_k ew[b,k] * (x[b,:] @ W[k,o,:] + bias[k,o])

Sharding: EXPERT-parallel (E == n_cores == 8). Core k computes its
expert's contribution z_k[o,b] = ew[:,k] * (W[k] @ x.T + bias[k]) for ALL
512 rows; the host sums the 8 partial outputs during unshard. This cuts
per-core HBM traffic from ~18.7 MB (data-parallel: every core streams the
whole weight tensor) to ~4 MB (W[k] 2 MB bf16 + x 1 MB bf16 + y 1 MB bf16
out), leaving the kernel PE-bound at the 64-matmul/core floor.

Layout ([o on partitions, b on free]):
  - out bank oc (8 PSUM banks [128, 512] fp32): z[oc*128+ol, b], accumulated
    over 8 i-tiles: matmul(lhsT=W tile [i, o], rhs=xT tile [i, b]).
  - bias[o] is per-partition -> folded into the PSUM eviction on ScalarE
    (activation Identity with per-partition bias AP).
  - ew[b] varies along free -> host-replicated broadcast tile [128, 512]
    fp32, applied by one DVE tensor_tensor per bank (fp32 -> bf16 out).
  - x and W arrive bf16 in ONE packed dram tensor `wx`, chunked so every
    matmul's two operands are covered by a single earlier DMA on the same
    queue (the walrus build accepts at most ONE sync wait per
    instruction): chunk 0 carries [xT tile 0 | W chunk 0] so the first
    matmul has one dep; later x tiles arrive before the W chunk that
    first needs them.
  - DMA order trickles x tiles during the (cold, HAM-throttled) bank-0
    matmuls and lands W chunk oc before PE finishes bank oc-1, so PE
    never stalls after bank 1; per-bank eviction + y DMA overlap PE work
    on later banks.
"""

import numpy as np

B, E, IN, OUT = 512, 8, 1024, 1024
NCORES = 8
P = 128
NIT = IN // P    # 8 i-tiles (contraction chunks)
NOC = OUT // P   # 8 o-chunks (one PSUM bank each)
NB = B           # moving free dim: all 512 rows in one matmul

# wx (bf16, [128, 12288]) column layout:
#   [0:512)      xp_0   (xT i-tile 0)
#   [512:1536)   wp_0   (W chunk oc=0: tiles (0, ib=0..7))
#   [1536:5120)  xp_1..xp_7, 512 cols each
#   [5120:12288) wp_1..wp_7, 1024 cols each
XP0 = 0
WP0 = 512
XPR = 1536
WPR = 5120
WX_COLS = 12288

# sm (fp32, [128, 520]): cols 0..7 bias per-partition (biasP[p, oc] =
# bias[k, oc*128+p]); cols 8..519 ew[:, k] replicated across partitions.
SM_BIAS = 0
SM_EW = 8
SM_COLS = 520

_compiled = None


def _xp_col(ib):
    return XP0 if ib == 0 else XPR + (ib - 1) * 512


def _wp_col(oc, ib):
    return (WP0 if oc == 0 else WPR + (oc - 1) * 1024) + ib * P


def _patch_drain_split():
    """The walrus build in this container rejects any instruction carrying
    more than one sync wait, including the kernel-tail Drain that
    TileContext emits with one wait per active semaphore. Split it into a
    sequence of single-wait drains (sequencer-FIFO keeps them ordered;
    the set of waits is identical)."""
    import concourse.tile as tile_mod

    if getattr(tile_mod.TileContext, "_drain_split_patched", False):
        return
    from concourse.tile_sem_assignment import N_PROCS
    from concourse.vector_clock import ScopedClock, VectorClock

    def _drain_and_barrier(self, tick_clock, wait_clock):
        gc = tick_clock.global_clock
        for p in range(N_PROCS):
            t = gc[p]
            if t <= 0:
                continue
            ticks = [0] * N_PROCS
            ticks[p] = t
            di = self.nc.sync.drain()
            wait_clock.add_sem_waits(
                di.ins, ScopedClock({None: VectorClock(ticks)})
            )
        self.nc.all_engine_barrier()
        assert self.sems is not None
        popped = self.nc._tile_sem_poison_stack.pop()
        assert popped is self._sem_poison
        self.nc.clear_and_free_semaphores(list(self.sems.allocated().values()))
        self.nc.all_engine_barrier()

    tile_mod.TileContext._drain_and_barrier = _drain_and_barrier
    tile_mod.TileContext._drain_split_patched = True


def _build():
    import concourse.bass as bass
    import concourse.mybir as mybir
    import concourse.tile as tile

    _patch_drain_split()

    f32 = mybir.dt.float32
    bf16 = mybir.dt.bfloat16

    nc = bass.Bass()
    wx_d = nc.dram_tensor("wx", [P, WX_COLS], bf16, kind="ExternalInput")
    sm_d = nc.dram_tensor("sm", [P, SM_COLS], f32, kind="ExternalInput")
    y_d = nc.dram_tensor("y", [P, NOC * NB], bf16, kind="ExternalOutput")

    with tile.TileContext(nc) as tc:
        with (
            tc.tile_pool(name="sb", bufs=1) as sb,
            tc.tile_pool(name="psum", bufs=1, space="PSUM") as psum,
        ):
            wx = sb.tile([P, WX_COLS], bf16)
            sm = sb.tile([P, SM_COLS], f32)
            scratch = sb.tile([P, 2], f32)
            tmps = [sb.tile([P, NB], f32, name=f"tmp{oc}") for oc in range(NOC)]
            y_sb = sb.tile([P, NOC * NB], bf16)
            pss = [psum.tile([P, NB], f32, name=f"ps{oc}") for oc in range(NOC)]

            def dma_wx(c0, c1):
                nc.sync.dma_start(wx[:, c0:c1], wx_d[:, c0:c1])

            # D0: [xp_0 | wp_0] — the first matmul's two operands in ONE
            # DMA (single sync wait). Then x tiles 1-4, W chunk 1, x tiles
            # 5-7, W chunks 2-4, sm (small operands, needed first by the
            # bank-0 eviction), W chunks 5-7.
            dma_wx(0, XPR)
            for ib in (1, 2, 3, 4):
                c = _xp_col(ib)
                dma_wx(c, c + 512)
            c = _wp_col(1, 0)
            dma_wx(c, c + NIT * P)
            for ib in (5, 6, 7):
                c = _xp_col(ib)
                dma_wx(c, c + 512)
            for oc in (2, 3, 4):
                c = _wp_col(oc, 0)
                dma_wx(c, c + NIT * P)
            nc.sync.dma_start(sm[:], sm_d[:])
            for oc in (5, 6, 7):
                c = _wp_col(oc, 0)
                dma_wx(c, c + NIT * P)

            # Absorb the sm-DMA wait on each consumer engine with a tiny
            # op, so the per-bank evictions carry only their producer's
            # wait (one-sync-wait walrus limit).
            nc.scalar.copy(scratch[:, 0:1], sm[:, 0:1])
            nc.vector.tensor_copy(scratch[:, 1:2], sm[:, SM_EW:SM_EW + 1])

            for oc in range(NOC):
                ps = pss[oc]
                for ib in range(NIT):
                    wc = _wp_col(oc, ib)
                    xc = _xp_col(ib)
                    nc.tensor.matmul(
                        ps[:],
                        wx[:, wc:wc + P],
                        wx[:, xc:xc + NB],
                        start=(ib == 0),
                        stop=(ib == NIT - 1),
                    )
                # Eviction: tmp = ps + bias[o] (per-partition, ScalarE),
                # y = tmp * ew[b] (broadcast row, DVE) -> bf16, then DMA.
                nc.scalar.add(tmps[oc][:], ps[:], sm[:, SM_BIAS + oc:SM_BIAS + oc + 1])
                nc.vector.tensor_tensor(
                    y_sb[:, oc * NB:(oc + 1) * NB],
                    tmps[oc][:],
                    sm[:, SM_EW:SM_EW + NB],
                    mybir.AluOpType.mult,
                )
                # SWDGE (gpsimd) queue: its DMASW sem lanes are separate
                # from the 8 HWDGE lanes the 16 input DMAs cycle through,
                # so these 8 carry only their DVE data wait (the HWDGE
                # lane-reuse guard would be a second sync wait -> rejected
                # by this walrus build).
                nc.gpsimd.dma_start(
                    y_d[:, oc * NB:(oc + 1) * NB],
                    y_sb[:, oc * NB:(oc + 1) * NB],
                )

    return nc


def _get_compiled():
    global _compiled
    if _compiled is None:
        _compiled = _build()
    return _compiled


_wp_cache = None


def _make_in_maps(x, expert_weights, weight, bias):
    global _wp_cache
    import ml_dtypes

    bf = ml_dtypes.bfloat16
    if _wp_cache is None or _wp_cache[0] is not weight:
        w = np.asarray(weight, dtype=np.float32)
        # wp[k][p, (oc, ib, ol)] = W[k, oc*128+ol, ib*128+p]
        wp = np.ascontiguousarray(
            w.reshape(E, NOC, P, NIT, P).transpose(0, 4, 1, 3, 2)
        ).astype(bf).reshape(E, P, NOC * NIT * P)
        _wp_cache = (weight, wp)
    wp = _wp_cache[1]

    x = np.asarray(x, dtype=np.float32)
    ew = np.asarray(expert_weights, dtype=np.float32)
    bias = np.asarray(bias, dtype=np.float32)

    # xp[p, (ib, b)] = x[b, ib*128+p]
    xp = (
        np.ascontiguousarray(x.T.reshape(NIT, P, B).transpose(1, 0, 2))
        .astype(bf)
        .reshape(P, NIT * B)
    )

    in_maps = []
    for k in range(NCORES):
        wx = np.empty((P, WX_COLS), dtype=bf)
        for ib in range(NIT):
            c = _xp_col(ib)
            wx[:, c:c + B] = xp[:, ib * B:(ib + 1) * B]
        for oc in range(NOC):
            c = _wp_col(oc, 0)
            wx[:, c:c + NIT * P] = wp[k, :, oc * NIT * P:(oc + 1) * NIT * P]
        sm = np.empty((P, SM_COLS), dtype=np.float32)
        sm[:, SM_BIAS:SM_BIAS + NOC] = bias[k].reshape(NOC, P).T
        sm[:, SM_EW:SM_EW + B] = np.broadcast_to(ew[:, k], (P, B))
        in_maps.append({"wx": wx, "sm": sm})
    return in_maps


def kernel(x, expert_weights, weight, bias, _trace=False):
    from concourse.bass_utils import run_bass_kernel_spmd

    nc = _get_compiled()
    in_maps = _make_in_maps(x, expert_weights, weight, bias)
    res = run_bass_kernel_spmd(
        nc, in_maps, core_ids=list(range(NCORES)), trace=_trace
    )
    # y_core[p, oc*512 + b] = z_k[oc*128+p, b]; unshard = sum over experts,
    # then [o, b] -> [b, o].
    acc = np.zeros((P, NOC * NB), dtype=np.float32)
    for r in res.results:
        acc += np.asarray(r["y"], dtype=np.float32)
    y = (
        acc.reshape(P, NOC, NB)
        .transpose(1, 0, 2)
        .reshape(OUT, B)
        .T.copy()
    )
    if _trace:
        return y, res
    return y
